# revision 15
# baseline (speedup 1.0000x reference)
"""Causal multi-head attention (B=2, T=2048, C=1024, H=16, HD=64) on 8 TRN2 cores.

Sharding: core = batch*4 + group; each core computes 4 heads of one batch
(Megatron-style tensor parallel over heads, data parallel over batch).
Each core: QKV projection for its heads -> causal attention -> partial output
projection over its 256 channels. Host sums the 4 partials per batch.

On-chip layout is fully "transposed": q/k kept as [channels, tokens] so the
scores matmul produces S^T [k, q]; softmax denominators come from packed M=1
ones-matmuls; the P@V matmul is column-packed 2 heads per PSUM bank; and the
output projection consumes y^T as the stationary operand, producing out in
natural [token, channel] layout. Phases are interleaved per 512-token tile so
TensorE (projections) and ScalarE (exp) overlap.
"""

import numpy as np
import ml_dtypes

import concourse.bass as bass
import concourse.bacc as bacc
import concourse.mybir as mybir
import concourse.tile as tile
from concourse.masks import make_upper_triangular

B, T, C = 2, 2048, 1024
H, HD = 16, 64
HPC = 4                      # heads per core
CPC = HPC * HD               # channels per core (256)
N_CORES = 8
TQ = 512                     # q tile (moving dim)
TK = 128                     # k tile (contraction chunk)
NQT = T // TQ                # 4
NKT = T // TK                # 16
NCC = C // 128               # 8 contraction chunks over C

BF = mybir.dt.bfloat16
F32 = mybir.dt.float32
Exp = mybir.ActivationFunctionType.Exp
Copy = mybir.ActivationFunctionType.Copy

_CACHE = {}


def _emit(tc, xt, wqk, wv, wo, out):
    nc = tc.nc
    ctx_pools = []

    def pool(name, bufs, space="SBUF"):
        p = tc.tile_pool(name=name, bufs=bufs, space=space)
        ctx_pools.append(p)
        return p.__enter__()

    const_p = pool("const", 1)
    w_p = pool("w", 1)
    x_p = pool("x", 2)
    qk_p = pool("qk", 1)
    v_p = pool("v", 1)
    y_p = pool("y", 1)
    e_p = pool("e", 4)
    rec_p = pool("rec", 2)
    rb_p = pool("rb", 2)
    o_p = pool("o", 3)
    # PSUM budget (8 banks of [128, 2KB]):
    #   ps_s: scores pair tiles [128,1024] (2 banks) x2   = 4
    #   ps_mm: QKV / V / out-proj tiles [128,512] x2      = 2
    #   ps_y: PV accumulator [128,512] (pair-sequential)  = 1
    #   ps_d: denominators rows 0/64; reused for bcast    = 1
    ps_s = pool("ps_s", 2, space="PSUM")
    ps_mm = pool("ps_mm", 2, space="PSUM")
    ps_y = pool("ps_y", 1, space="PSUM")
    ps_d = pool("ps_d", 1, space="PSUM")

    # ---- constants / weights ----
    mask = const_p.tile([128, 128], BF, tag="mask")
    make_upper_triangular(nc, mask[:, :], val=1.0, diag=True)
    ones_r = const_p.tile([1, 64], BF, tag="ones_r")
    nc.gpsimd.memset(ones_r[:, :], 1.0)
    ones_c = const_p.tile([128, 1], BF, tag="ones_c")
    nc.gpsimd.memset(ones_c[:, :], 1.0)

    wqk_sb = w_p.tile([128, NCC, 512], BF, tag="wqk")
    nc.sync.dma_start(wqk_sb[:, :, :], wqk.rearrange("(cc p) j -> p cc j", p=128))
    wv_sb = w_p.tile([128, NCC, CPC], BF, tag="wv")
    nc.sync.dma_start(wv_sb[:, :, :], wv.rearrange("(cc p) j -> p cc j", p=128))
    wo_sb = w_p.tile([128, 2, C], BF, tag="wo")
    nc.sync.dma_start(wo_sb[:, :, :], wo.rearrange("(cc p) d -> p cc d", p=128))

    # persistent activations: [channel, token] layout, bf16
    qA = qk_p.tile([128, T], BF, tag="qA")   # rows 0:64 head0, 64:128 head1
    qB = qk_p.tile([128, T], BF, tag="qB")   # heads 2,3
    kA = qk_p.tile([128, T], BF, tag="kA")
    kB = qk_p.tile([128, T], BF, tag="kB")
    v_sb = v_p.tile([128, NKT, HPC, 64], BF, tag="v")  # [t, kt, head, ch]
    yA = y_p.tile([128, T], BF, tag="yA")
    yB = y_p.tile([128, T], BF, tag="yB")

    xt_r = xt.rearrange("(cc p) t -> p cc t", p=128)
    qk_dst = (qA, qB, kA, kB)

    for tt in range(NQT):
        # ---- QKV projections for token tile tt ----
        x_sb = x_p.tile([128, NCC, TQ], BF, tag="x", name=f"x_{tt}")
        nc.sync.dma_start(x_sb[:, :, :], xt_r[:, :, tt * TQ:(tt + 1) * TQ])
        # Q,K: out^T [j, t] = wqk^T.T @ x^T ; stationary weights, moving x
        for jt in range(4):
            ps = ps_mm.tile([128, TQ], F32, tag="mm", name=f"qkps_{tt}_{jt}")
            for cc in range(NCC):
                nc.tensor.matmul(
                    ps[:, :],
                    wqk_sb[:, cc, jt * 128:(jt + 1) * 128],
                    x_sb[:, cc, :],
                    start=(cc == 0), stop=(cc == NCC - 1),
                )
            nc.vector.tensor_copy(qk_dst[jt][:, tt * TQ:(tt + 1) * TQ], ps[:, :])
        # V: out [t, vch] = x^T.T @ wv^T ; stationary x slice, moving weights
        for sub in range(4):
            kt = tt * 4 + sub
            psv = ps_mm.tile([128, CPC], F32, tag="mm", name=f"vps_{kt}")
            for cc in range(NCC):
                nc.tensor.matmul(
                    psv[:, :],
                    x_sb[:, cc, sub * 128:(sub + 1) * 128],
                    wv_sb[:, cc, :],
                    start=(cc == 0), stop=(cc == NCC - 1),
                )
            nc.vector.tensor_copy(
                v_sb[:, kt, :, :],
                psv[:, :].rearrange("p (h d) -> p h d", h=HPC),
            )

        # ---- causal attention for q tile qt = tt (S^T layout) ----
        qt = tt
        nkt = 4 * (qt + 1)
        qs = slice(qt * TQ, (qt + 1) * TQ)
        for pair, (q_t, k_t, y_t) in enumerate(((qA, kA, yA), (qB, kB, yB))):
            psy = ps_y.tile([128, TQ], F32, tag="y", name=f"psy_{qt}_{pair}")
            psd = ps_d.tile([128, TQ], F32, tag="d", name=f"psd_{qt}_{pair}")
            for kt in range(nkt):
                d = kt - qt * 4  # >=0 -> diagonal region
                ks = slice(kt * TK, (kt + 1) * TK)
                pss = ps_s.tile([128, 2, TQ], F32, tag="s",
                                name=f"pss_{qt}_{kt}_{pair}")
                e_sb = e_p.tile([128, 2, TQ], BF, tag="e",
                                name=f"e_{qt}_{kt}_{pair}")
                for hh in range(2):
                    lo, hi = hh * 64, hh * 64 + 64
                    nc.tensor.matmul(pss[:, hh, :], k_t[lo:hi, ks], q_t[lo:hi, qs],
                                     start=True, stop=True)
                if d < 0:
                    nc.scalar.activation(e_sb[:, :, :], pss[:, :, :], Exp, scale=0.125)
                else:
                    if d > 0:
                        nc.gpsimd.memset(e_sb[:, :, 0:d * 128], 0.0)
                    nc.scalar.activation(e_sb[:, :, d * 128:], pss[:, :, d * 128:],
                                         Exp, scale=0.125)
                    for hh in range(2):
                        nc.vector.tensor_mul(
                            e_sb[:, hh, d * 128:(d + 1) * 128],
                            e_sb[:, hh, d * 128:(d + 1) * 128],
                            mask[:, :])
                first, last = kt == 0, kt == nkt - 1
                for hh in range(2):
                    h = pair * 2 + hh
                    nc.tensor.matmul(
                        psy[hh * 64:hh * 64 + 64, :],
                        v_sb[:, kt, h, :],
                        e_sb[:, hh, :],
                        start=first, stop=last, skip_group_check=True,
                    )
                for hh in range(2):
                    nc.tensor.matmul(
                        psd[hh * 64:hh * 64 + 1, :],
                        ones_c[:, :],
                        e_sb[:, hh, :],
                        start=first, stop=last, skip_group_check=True,
                    )
            # ---- softmax normalization + y^T ----
            # reciprocals of the two denominator rows, then rank-1 broadcast
            # back into the same psd bank (rows consumed by then)
            recs = []
            for hh in range(2):
                rec = rec_p.tile([1, TQ], BF, tag="rec",
                                 name=f"rec_{qt}_{pair}_{hh}")
                with nc.allow_low_precision(reason="bf16 recip row for bcast"):
                    nc.vector.reciprocal(rec[:, :], psd[hh * 64:hh * 64 + 1, :])
                recs.append(rec)
            for hh in range(2):
                nc.tensor.matmul(psd[hh * 64:hh * 64 + 64, :], ones_r[:, :],
                                 recs[hh][:, :], start=True, stop=True,
                                 skip_group_check=True)
            rb = rb_p.tile([128, TQ], F32, tag="rb", name=f"rb_{qt}_{pair}")
            nc.scalar.activation(rb[:, :], psd[:, :], Copy)
            nc.vector.tensor_mul(y_t[:, qs], psy[:, :], rb[:, :])

        # ---- output projection for token tile tt ----
        for sub in range(4):
            tt2 = tt * 4 + sub
            t2 = slice(tt2 * 128, (tt2 + 1) * 128)
            for dh in range(2):
                ps = ps_mm.tile([128, 512], F32, tag="mm", name=f"ops_{tt2}_{dh}")
                nc.tensor.matmul(ps[:, :], yA[:, t2],
                                 wo_sb[:, 0, dh * 512:(dh + 1) * 512],
                                 start=True, stop=False)
                nc.tensor.matmul(ps[:, :], yB[:, t2],
                                 wo_sb[:, 1, dh * 512:(dh + 1) * 512],
                                 start=False, stop=True)
                o_sb = o_p.tile([128, 512], F32, tag="o", name=f"o_{tt2}_{dh}")
                nc.vector.tensor_copy(o_sb[:, :], ps[:, :])
                nc.sync.dma_start(out[t2, dh * 512:(dh + 1) * 512], o_sb[:, :])

    for p in reversed(ctx_pools):
        p.__exit__(None, None, None)


def build():
    if "nc" in _CACHE:
        return _CACHE["nc"]
    nc = bacc.Bacc("TRN2", target_bir_lowering=False, debug=False)
    xt = nc.dram_tensor("xt", [C, T], BF, kind="ExternalInput").ap()
    wqk = nc.dram_tensor("wqk", [C, 512], BF, kind="ExternalInput").ap()
    wv = nc.dram_tensor("wv", [C, CPC], BF, kind="ExternalInput").ap()
    wo = nc.dram_tensor("wo", [CPC, C], BF, kind="ExternalInput").ap()
    out = nc.dram_tensor("out", [T, C], F32, kind="ExternalOutput").ap()
    with tile.TileContext(nc) as tc:
        _emit(tc, xt, wqk, wv, wo, out)
    nc.compile()
    _CACHE["nc"] = nc
    return nc


def shard_inputs(x, w_qkv, w_o):
    bf16 = ml_dtypes.bfloat16
    x = np.asarray(x, np.float32)
    w_qkv = np.asarray(w_qkv, np.float32)
    w_o = np.asarray(w_o, np.float32)
    xt_b = [np.ascontiguousarray(x[b].T).astype(bf16) for b in range(B)]
    in_maps = []
    for core in range(N_CORES):
        b, g = divmod(core, 4)
        r0, r1 = g * CPC, (g + 1) * CPC
        wqk_c = np.ascontiguousarray(
            np.concatenate([w_qkv[r0:r1], w_qkv[C + r0:C + r1]], axis=0).T
        ).astype(bf16)
        wv_c = np.ascontiguousarray(w_qkv[2 * C + r0:2 * C + r1].T).astype(bf16)
        wo_c = np.ascontiguousarray(w_o[:, r0:r1].T).astype(bf16)
        in_maps.append({"xt": xt_b[b], "wqk": wqk_c, "wv": wv_c, "wo": wo_c})
    return in_maps


def run(in_maps, **kwargs):
    from concourse.bass_utils import run_bass_kernel_spmd
    nc = build()
    return run_bass_kernel_spmd(nc, in_maps, list(range(N_CORES)), **kwargs)


def kernel(x, w_qkv, w_o):
    res = run(shard_inputs(x, w_qkv, w_o))
    out = np.zeros((B, T, C), np.float32)
    for core in range(N_CORES):
        b = core // 4
        out[b] += res.results[core]["out"]
    return out
